# revision 58
# baseline (speedup 1.0000x reference)
import math

import numpy as np

# nn_DescLayer, period-sharded design.
#
# y[t,i] = res[t,i] + sum_{j,g} P[i,j,g] cos(2pi k_t / per[i,j,g]) xp[t,j]
#        = res[t,i] + sum_j W[k_t, i, j] xp[t,j],
#   W[k,i,j] = sum_g P[i,j,g] cos(k * w_g),  w_g = 2pi/per[i,j,g].
#
# Sharding: core c owns output columns i in {c, c+8, ..., c+56}; every core
# processes all 1024 tokens (no cross-core communication).
#
# Key reduction (8x fewer trig evals): for per >= ~240, linearize over g
# around what = mean_g(w_g):
#   W[k,i,j] ~= cos(k*what)*A[i,j] - k*sin(k*what)*B[i,j],
#   A = sum_g P, B = sum_g P*(w_g - what).
# Exact per-g evaluation (with mod-1 range reduction) only for the first
# EXJ j's of i_loc=0 (smallest periods). Validated: end-to-end rel err
# ~4e-4 vs fp64 reference.
#
# Tables are computed with k on partitions (k = kc*128 + p, kc=0..3), ij on
# the free dim, so the ScalarE Sin activation with per-partition scale=k
# batches 128 k-values per instruction. The per-token "gather" W[k_t] is a
# PE one-hot matmul: stationary U[k, t] (host-built one-hot), moving
# W2[k, ij] fp16, accumulated over the 4 k-chunks into PSUM [t, ij] per
# 128-token group. Combine = broadcast-mult by xp (stride-0 AP views) +
# free-dim reduce over j.

B, S, D, NB = 2, 512, 64, 8
N_CORES = 8
NT = B * S  # 1024 tokens, all on every core
NK = 512  # k values
NKC = 4  # k chunks of 128
IJ = 512  # (i_loc, j) entries per core
SMJ = 64  # i_loc=0 columns (linearized via mod-1 reduction)
EXJ = 20  # exact j's (i_loc=0, j<EXJ)
EXC = EXJ * NB  # 240 exact flat columns
REGC = IJ - SMJ  # 448 regular columns (direct LUT)
NTG = NT // 128  # 8 token groups
LN_EPS = 1e-5
RND_C = 12582912.0  # 1.5*2^23: (u+C)-C == round-to-nearest(u) in f32
TWO_PI = 2.0 * math.pi

_CACHE = {}


def _split_waits(nc, maxw=1):
    """This walrus build rejects instructions carrying more than one sem
    wait. Hoist excess waits onto same-engine NoOps placed immediately
    before the instruction (same engine stream => executes first)."""
    import bass_rust
    import concourse.mybir as mybir

    ctr = [0]
    for f in nc.m.functions:
        for b in f.blocks:
            new_insts = []
            changed = False
            for inst in b.instructions:
                si = inst.sync_info
                waits = list(si.on_wait) if si and si.on_wait else []
                if len(waits) > maxw:
                    keep = waits[-maxw:]
                    extra = waits[:-maxw]
                    for i0 in range(0, len(extra), maxw):
                        ctr[0] += 1
                        nop = bass_rust.InstNoOp(
                            name=f"I-waitsplit-{ctr[0]}",
                            engine=inst.engine,
                            text_hint="waitsplit",
                            sync_info=mybir.SyncInfo(
                                on_wait=extra[i0 : i0 + maxw], on_update=[]
                            ),
                        )
                        new_insts.append(nop)
                    si.on_wait = keep
                    changed = True
                new_insts.append(inst)
            if changed:
                b.instructions = new_insts


def _build_program():
    import concourse.bass as bass
    import concourse.mybir as mybir
    from concourse.tile import TileContext
    from concourse.vector_clock import ScopedClock, VectorClock

    # walrus also rejects the multi-wait Tile tail drain; spread the waits
    # over SP nops (1 each), then issue a bare drain.
    def _drain_and_barrier(self, tick_clock, wait_clock):
        nc = self.nc
        gc = tick_clock.global_clock
        n = len(gc)
        for i in range(n):
            tick = gc[i]
            if tick <= 0:
                continue
            vec = [0] * n
            vec[i] = tick
            nop_inst = nc.sync.nop(nofuse=True, hint=f"drain_wait_{i}")
            wait_clock.add_sem_waits(
                nop_inst.ins, ScopedClock({None: VectorClock(vec)})
            )
        nc.sync.drain()
        nc.all_engine_barrier()
        assert self.sems is not None
        popped = nc._tile_sem_poison_stack.pop()
        assert popped is self._sem_poison
        nc.clear_and_free_semaphores(list(self.sems.allocated().values()))
        nc.all_engine_barrier()

    TileContext._drain_and_barrier = _drain_and_barrier

    f32 = mybir.dt.float32
    f16 = mybir.dt.float16
    f8 = mybir.dt.float8e4
    i32 = mybir.dt.int32
    AF = mybir.ActivationFunctionType
    OP = mybir.AluOpType
    AX = mybir.AxisListType

    nc = bass.Bass()
    X = nc.declare_dram_parameter("x", [128, NTG * D], f16, isOutput=False)
    U = nc.declare_dram_parameter("u1h", [128, NKC * NT], f8, isOutput=False)
    # kcols folded into the tail of bc32 (one fewer DMA; the standalone
    # 128x32B transfer was the straggler gating first compute)
    NB32 = REGC + EXC + SMJ + D + 8 + 1 + 2 * NKC
    NB16 = 2 * IJ + EXC
    B32 = nc.declare_dram_parameter("bc32", [128, NB32], f32, isOutput=False)
    B16 = nc.declare_dram_parameter("bc16", [128, NB16], f16, isOutput=False)
    MG = nc.declare_dram_parameter("mgt", [D, D], f16, isOutput=False)
    RG = nc.declare_dram_parameter("rgt", [D, 8], f16, isOutput=False)
    IDM = nc.declare_dram_parameter("idm", [128, 128], f16, isOutput=False)
    Y = nc.declare_dram_parameter("y", [128, NTG * 8], f32, isOutput=True)

    with TileContext(nc) as tc:
        with (
            tc.tile_pool(name="const", bufs=1) as cp,
            tc.tile_pool(name="tab", bufs=2) as tp,
            tc.tile_pool(name="wrow", bufs=2, space="PSUM") as wrp,
            tc.tile_pool(name="pprep", bufs=1, space="PSUM") as pp,
        ):
            # ---------------- input DMAs ----------------
            # One dma_start maps to ONE DMA queue (per-partition descriptors
            # serialize on it) and its issue costs ~600ns on the engine
            # sequencer. Per-engine queues are FIFO, so issue order = arrival
            # order: bc32 (gates the whole trig phase) first, then xs (LN),
            # bc16 (w2 assembly), and the fat u1h (only needed at matmul
            # time ~late) LAST.
            bc32 = cp.tile([128, NB32], f32, tag="bc32")
            bc16 = cp.tile([128, NB16], f16, tag="bc16")
            xs = cp.tile([128, NTG, D], f16, tag="xs")
            xv = X[:].rearrange("p (t j) -> p t j", j=D)
            u1h = cp.tile([128, NKC * NT], f8, tag="u1h")
            nc.sync.dma_start(out=bc32[0:32, :], in_=B32[0:32, :])
            nc.scalar.dma_start(out=bc32[32:64, :], in_=B32[32:64, :])
            nc.gpsimd.dma_start(out=bc32[64:96, :], in_=B32[64:96, :])
            nc.sync.dma_start(out=bc32[96:128, :], in_=B32[96:128, :])
            # ACT table preload: a dummy Sin on a memset tile forces the
            # 1.3us ACT_TABLE_LOAD to run now, off the critical path,
            # instead of right before the first real activation.
            dummy = cp.tile([128, 1], f32, tag="dummy")
            nc.gpsimd.memset(dummy[:], 0.0)
            dumo = cp.tile([128, 1], f32, tag="dumo")
            nc.scalar.activation(dumo[:], dummy[:], AF.Sin, bias=0.0, scale=1.0)
            nc.sync.dma_start(out=xs[0:64, :, :], in_=xv[0:64])
            nc.gpsimd.dma_start(out=xs[64:128, :, :], in_=xv[64:128])
            nc.gpsimd.dma_start(out=bc16[0:64, :], in_=B16[0:64, :])
            nc.gpsimd.dma_start(out=bc16[64:128, :], in_=B16[64:128, :])
            idm = cp.tile([128, 128], f16, tag="idm")
            nc.sync.dma_start(out=idm[:], in_=IDM[:])
            mgt = cp.tile([D, D], f16, tag="mgt")
            nc.sync.dma_start(out=mgt[:], in_=MG[:])
            rgt = cp.tile([D, 8], f16, tag="rgt")
            nc.sync.dma_start(out=rgt[:], in_=RG[:])
            nc.scalar.dma_start(out=u1h[0:64, :], in_=U[0:64, :])
            nc.gpsimd.dma_start(out=u1h[64:128, :], in_=U[64:128, :])
            whatc = bc32[:, 0:REGC]
            vin = bc32[:, REGC : REGC + EXC + SMJ]
            mbet = bc32[:, REGC + EXC + SMJ : REGC + EXC + SMJ + D]
            rbet = bc32[:, REGC + EXC + SMJ + D : REGC + EXC + SMJ + D + 8]
            hpc = bc32[:, REGC + EXC + SMJ + D + 8 : REGC + EXC + SMJ + D + 9]
            kcols = bc32[:, REGC + EXC + SMJ + D + 9 : REGC + EXC + SMJ + D + 9 + 2 * NKC]
            acol = bc16[:, 0:IJ]
            bcol = bc16[:, IJ : 2 * IJ]
            p0b = bc16[:, 2 * IJ : 2 * IJ + EXC]

            # ---------------- V/U frac chains (vector, first) -------------
            NV = EXC + SMJ
            u4v = tp.tile([128, NKC, NV], f32, tag="u4v")
            for kc in range(NKC):
                nc.vector.tensor_scalar(
                    u4v[:, kc, :], vin, kcols[:, kc : kc + 1], 0.25, OP.mult, OP.add
                )
            r4v = tp.tile([128, NKC, NV], f32, tag="r4v")
            nc.vector.tensor_scalar(r4v[:], u4v[:], RND_C, RND_C, OP.add, OP.subtract)
            f4v = tp.tile([128, NKC, NV], f32, tag="f4v")
            nc.vector.tensor_tensor(f4v[:], u4v[:], r4v[:], OP.subtract)
            u4u = tp.tile([128, NKC, SMJ], f32, tag="u4u")
            for kc in range(NKC):
                nc.vector.tensor_scalar(
                    u4u[:, kc, :], vin[:, EXC:], kcols[:, kc : kc + 1], None, OP.mult
                )
            r4u = tp.tile([128, NKC, SMJ], f32, tag="r4u")
            nc.vector.tensor_scalar(r4u[:], u4u[:], RND_C, RND_C, OP.add, OP.subtract)
            f4u = tp.tile([128, NKC, SMJ], f32, tag="f4u")
            nc.vector.tensor_tensor(f4u[:], u4u[:], r4u[:], OP.subtract)

            # ---------------- scalar acts: tables --------------------------
            # order: kc0/kc1 regular, then the small-i (SMJ) and vcos acts
            # (unblocks i1/w2/pc0 for kc0-1 early), then kc2/kc3 regular.
            sinT = tp.tile([128, NKC, IJ], f16, tag="sinT")
            cosT = tp.tile([128, NKC, IJ], f16, tag="cosT")
            vcos = tp.tile([128, NKC, EXC], f16, tag="vcos")
            for kc in range(2):
                k2p = kcols[:, kc : kc + 1]
                nc.scalar.activation(
                    sinT[:, kc, SMJ:], whatc, AF.Sin, bias=0.0, scale=k2p
                )
                nc.scalar.activation(
                    cosT[:, kc, SMJ:], whatc, AF.Sin, bias=hpc, scale=k2p
                )
            nc.scalar.activation(
                cosT[:, :, :SMJ], f4v[:, :, EXC:], AF.Sin, bias=0.0, scale=TWO_PI
            )
            nc.scalar.activation(
                sinT[:, :, :SMJ], f4u[:], AF.Sin, bias=0.0, scale=TWO_PI
            )
            nc.scalar.activation(
                vcos[:], f4v[:, :, :EXC], AF.Sin, bias=0.0, scale=TWO_PI
            )
            for kc in range(2, NKC):
                k2p = kcols[:, kc : kc + 1]
                nc.scalar.activation(
                    sinT[:, kc, SMJ:], whatc, AF.Sin, bias=0.0, scale=k2p
                )
                nc.scalar.activation(
                    cosT[:, kc, SMJ:], whatc, AF.Sin, bias=hpc, scale=k2p
                )

            # ---------------- LN stats -> rstd only ------------------------
            # centering is folded into mgt/rgt on the host (C = I - J/64
            # pre-multiplied), so we only need rstd[t].
            # xsq in fp16 on Vector (2x DVE mode; TTR opcode is rejected by
            # this walrus build)
            xsq = cp.tile([128, NTG, D], f16, tag="xsq")
            nc.vector.tensor_tensor(xsq[:], xs[:], xs[:], OP.mult)
            ssq = cp.tile([128, NTG], f32, tag="ssq")
            with nc.allow_low_precision(reason="fp16 squares, f32 accum"):
                nc.vector.tensor_reduce(ssq[:], xsq[:], AX.X, OP.add)
            rsum = cp.tile([128, NTG], f32, tag="rsum")
            nc.vector.tensor_reduce(rsum[:], xs[:], AX.X, OP.add)
            s2 = cp.tile([128, NTG], f32, tag="s2")
            nc.vector.tensor_tensor(s2[:], rsum[:], rsum[:], OP.mult)
            s2b = cp.tile([128, NTG], f32, tag="s2b")
            nc.vector.tensor_scalar(
                s2b[:], s2[:], 1.0 / (D * D), LN_EPS, OP.mult, OP.subtract
            )
            vep3 = cp.tile([128, NTG], f32, tag="vep3")
            nc.vector.scalar_tensor_tensor(
                vep3[:], ssq[:], 1.0 / D, s2b[:], OP.mult, OP.subtract
            )
            # rsqrt: quake seed + 1 Newton step
            tix = cp.tile([128, NTG], i32, tag="tix")
            nc.vector.tensor_scalar(
                tix[:], vep3[:].bitcast(i32), 1, -1, OP.arith_shift_right,
                OP.bitwise_xor,
            )
            yr = cp.tile([128, NTG], f32, tag="yr")
            nc.vector.tensor_scalar(
                yr[:].bitcast(i32), tix[:], 0x5F3759DF + 1, None, OP.add
            )
            for it in range(1):
                h = cp.tile([128, NTG], f32, tag=f"nwh_{it}")
                nc.vector.scalar_tensor_tensor(
                    h[:], vep3[:], 0.5, yr[:], OP.mult, OP.mult
                )
                t2 = cp.tile([128, NTG], f32, tag=f"nw2_{it}")
                nc.vector.tensor_tensor(t2[:], h[:], yr[:], OP.mult)
                t3 = cp.tile([128, NTG], f32, tag=f"nw3_{it}")
                nc.vector.tensor_scalar(t3[:], t2[:], 1.5, -1.0, OP.subtract, OP.mult)
                yn = cp.tile([128, NTG], f32, tag=f"nwy_{it}")
                nc.vector.tensor_tensor(yn[:], yr[:], t3[:], OP.mult)
                yr = yn
            rstd = yr

            # ---------------- xT + projections (PE, right after x DMA) -----
            # transposes run on RAW x; mgt/rgt carry centering+gamma.
            lnT_ps = pp.tile([D, NT], f16, tag="lnT_ps")
            for tg in range(NTG):
                nc.tensor.transpose(
                    lnT_ps[:, tg * 128 : (tg + 1) * 128], xs[:, tg, :], idm[:]
                )
            lnT = cp.tile([D, NT], f16, tag="lnT")
            nc.scalar.copy(lnT[:], lnT_ps[:])
            xp_ps = pp.tile([128, NTG, D], f32, tag="xp_ps")
            res_ps = pp.tile([128, NTG, 8], f32, tag="res_ps")
            for tg in range(NTG):
                sl = lnT[:, tg * 128 : (tg + 1) * 128]
                nc.tensor.matmul(xp_ps[:, tg, :], sl, mgt[:], start=True, stop=True)
                nc.tensor.matmul(res_ps[:, tg, :], sl, rgt[:], start=True, stop=True)
            # xp_tok = rstd*xp + mbeta, res_pre = rstd*res + rbeta
            rstd_bx = rstd[:].rearrange("p (t a) -> p t a", a=1)
            xps = cp.tile([128, NTG, D], f16, tag="xps")
            nc.vector.tensor_tensor(
                xps[:], xp_ps[:], rstd_bx.broadcast_to([128, NTG, D]), OP.mult
            )
            xp_tok = cp.tile([128, NTG, D], f16, tag="xp_tok")
            nc.vector.tensor_tensor(
                xp_tok[:], xps[:],
                mbet.rearrange("p (a j) -> p a j", a=1).broadcast_to([128, NTG, D]),
                OP.add,
            )
            rss = cp.tile([128, NTG, 8], f32, tag="rss")
            nc.vector.tensor_tensor(
                rss[:], res_ps[:], rstd_bx.broadcast_to([128, NTG, 8]), OP.mult
            )
            res_pre = cp.tile([128, NTG, 8], f32, tag="res_pre")
            nc.vector.tensor_tensor(
                res_pre[:], rss[:],
                rbet.rearrange("p (a i) -> p a i", a=1).broadcast_to([128, NTG, 8]),
                OP.add,
            )

            # ---------------- W2 [128k, (kc, ij)] fp16 ---------------------
            # W2 = cosT*A - (k/512)*sinT*(512*B); exact override on ij < EXJ.
            # All per-kc so the gather matmuls chase kc availability instead
            # of waiting on one monolithic 4us gpsimd op.
            w2 = tp.tile([128, NKC, IJ], f16, tag="w2")
            i1 = tp.tile([128, NKC, IJ], f16, tag="i1")
            pc0 = tp.tile([128, NKC, EXC], f16, tag="pc0")
            for kc in range(NKC):
                nc.vector.scalar_tensor_tensor(
                    i1[:, kc, :], sinT[:, kc, :], kcols[:, NKC + kc : NKC + kc + 1],
                    bcol, OP.mult, OP.mult,
                )
                # halved gpsimd ops: small Vector ops stall for the length
                # of whatever GpSimd instruction is in flight, so shorter
                # gpsimd instructions cut the stall tax on the i1/rstd chain
                for hb in range(2):
                    hs = slice(hb * (IJ // 2), (hb + 1) * (IJ // 2))
                    nc.gpsimd.tensor_tensor(
                        w2[:, kc, hs], cosT[:, kc, hs], acol[:, hs], OP.mult
                    )
                # subtract halves split across gpsimd and Vector: Vector
                # has a ~4us idle hole here (waiting on w2 for the gather),
                # and the fp16 TT runs at 2x there anyway
                hs0 = slice(0, IJ // 2)
                hs1 = slice(IJ // 2, IJ)
                nc.gpsimd.tensor_tensor(
                    w2[:, kc, hs0], w2[:, kc, hs0], i1[:, kc, hs0], OP.subtract
                )
                nc.vector.tensor_tensor(
                    w2[:, kc, hs1], w2[:, kc, hs1], i1[:, kc, hs1], OP.subtract
                )
                # pc0 on Vector (2x fp16 there, ~120ns) instead of gpsimd:
                # it was queued behind the w2 mult/sub work and starved the
                # EXJ reduce, leaving Vector idle waiting on it
                nc.vector.tensor_tensor(pc0[:, kc, :], vcos[:, kc, :], p0b, OP.mult)
                with nc.allow_low_precision(reason="8-way fp16 g-sum, ~4e-4"):
                    nc.vector.tensor_reduce(
                        w2[:, kc, :EXJ],
                        pc0[:, kc, :].rearrange("p (j g) -> p j g", g=NB),
                        AX.X, OP.add,
                    )

            # ---------------- one-hot gather on PE + combine ---------------
            # wrow copied PSUM->SBUF fp16 on Scalar so the DVE mult/reduce
            # run all-fp16-SBUF-packed (2x/4x DVE perf modes).
            nk_tok = cp.tile([128, NTG, 8], f16, tag="nk_tok")
            yout = cp.tile([128, NTG, 8], f32, tag="yout")
            for wv in range(4):
                wrow = wrp.tile([128, 2, IJ], f32, tag="wrow")
                for kc in range(NKC):
                    for tgi in range(2):
                        tg = wv * 2 + tgi
                        nc.tensor.matmul(
                            wrow[:, tgi, :],
                            u1h[:, kc * NT + tg * 128 : kc * NT + (tg + 1) * 128],
                            w2[:, kc, :],
                            start=(kc == 0),
                            stop=(kc == NKC - 1),
                        )
                wr16 = tp.tile([128, 2, IJ], f16, tag="wr16")
                nc.scalar.copy(wr16[:], wrow[:])
                m = tp.tile([128, 2, 8, D], f16, tag="m")
                nc.vector.tensor_tensor(
                    m[:],
                    wr16[:].rearrange("p t (i j) -> p t i j", j=D),
                    xp_tok[:, wv * 2 : wv * 2 + 2, :]
                    .rearrange("p t (a j) -> p t a j", a=1)
                    .broadcast_to([128, 2, 8, D]),
                    OP.mult,
                )
                # one 2x-mode fold (j 64->32) before the reduce: TENSOR_REDUCE
                # never gets the fp16 2x rate, TENSOR_TENSOR does
                mh = tp.tile([128, 2, 8, D // 2], f16, tag="mh")
                with nc.allow_low_precision(reason="fp16 combine reduce"):
                    nc.vector.tensor_tensor(
                        mh[:], m[:, :, :, 0 : D // 2], m[:, :, :, D // 2 : D],
                        OP.add,
                    )
                    nc.vector.tensor_reduce(
                        nk_tok[:, wv * 2 : wv * 2 + 2, :].rearrange(
                            "p a b -> p (a b)"
                        ),
                        mh[:].rearrange("p t i j -> p (t i) j"), AX.X, OP.add
                    )
                nc.vector.tensor_tensor(
                    yout[:, wv * 2 : wv * 2 + 2, :],
                    nk_tok[:, wv * 2 : wv * 2 + 2, :],
                    res_pre[:, wv * 2 : wv * 2 + 2, :],
                    OP.add,
                )

            # ---------------- output (3 stripes; none on gpsimd so its
            # software-DGE drain isn't gated by the y transfer)
            yv = yout[:].rearrange("p t i -> p (t i)")
            # 2 stripes, one per HWDGE engine: the 3-stripe layout serialized
            # two issues (~0.7us each) on sync at the very end of the kernel
            nc.sync.dma_start(out=Y[0:64, :], in_=yv[0:64])
            nc.scalar.dma_start(out=Y[64:128, :], in_=yv[64:128])

    _split_waits(nc)
    return nc


def kernel(x, k, M, R, P, gamma, beta, periods):
    from concourse.bass_utils import run_bass_kernel_spmd

    if "nc" not in _CACHE:
        _CACHE["nc"] = _build_program()
    nc = _CACHE["nc"]

    xf = np.ascontiguousarray(x, dtype=np.float32).reshape(NT, D)
    kf = np.ascontiguousarray(k, dtype=np.int64).reshape(NT)
    Mf = np.asarray(M, dtype=np.float64)
    Rf = np.asarray(R, dtype=np.float64)
    Pf = np.asarray(P, dtype=np.float64)
    gf = np.asarray(gamma, dtype=np.float64)
    bf = np.asarray(beta, dtype=np.float64)
    perf = np.asarray(periods, dtype=np.float64)

    # token-major x: token t -> (partition t%128, group t//128); fp16 halves
    # the DMA (stats/projection numerics still comfortably within budget)
    x_sb = xf.reshape(NTG, 128, D).transpose(1, 0, 2).reshape(128, NTG * D)
    x_sb = np.ascontiguousarray(x_sb.astype(np.float16))

    # one-hot U[k, t] blocked by k-chunk: u1h[p, kc*NT + t] = (k_t == kc*128+p)
    import ml_dtypes
    u1h = np.zeros((128, NKC * NT), dtype=np.float32)
    t_idx = np.arange(NT)
    u1h[kf % 128, (kf // 128) * NT + t_idx] = 1.0
    u1h = u1h.astype(ml_dtypes.float8_e4m3)

    # k scalar columns (appended to bc32): [0:4] = k, [4:8] = k/512
    pcol = np.arange(128, dtype=np.float64)
    kcols = np.zeros((128, 2 * NKC), dtype=np.float32)
    for kc in range(NKC):
        kcols[:, kc] = pcol + 128 * kc
        kcols[:, NKC + kc] = (pcol + 128 * kc) / 512.0

    idm = np.eye(128, dtype=np.float16)

    w = TWO_PI / perf  # (i, j, g)
    what = w.mean(-1)  # (i, j)
    A = Pf.sum(-1)  # (i, j)
    Bc = 512.0 * (Pf * (w - what[..., None])).sum(-1)  # (i, j), pre-scaled

    in_maps = []
    for c in range(N_CORES):
        i_set = np.arange(8) * 8 + c  # i_loc -> global i
        what_c = what[i_set]  # (8, 64)
        A_c = A[i_set]
        B_c = Bc[i_set]
        # regular cols: ij >= SMJ, i.e. i_loc >= 1
        whatc = what_c[1:].reshape(1, REGC).astype(np.float32)
        acol = A_c.reshape(1, IJ).astype(np.float16)
        bcol = B_c.reshape(1, IJ).astype(np.float16)
        # V/U path inputs: [exact 1/per (EXC) | smalli what/(2pi) (SMJ)]
        per0 = perf[i_set[0], :EXJ, :].reshape(EXC)  # exact periods
        invs = what_c[0] / TWO_PI  # (64,) cycles-per-k for i_loc = 0
        vin = np.concatenate([1.0 / per0, invs]).reshape(1, EXC + SMJ).astype(np.float32)
        p0 = Pf[i_set[0], :EXJ, :].reshape(1, EXC).astype(np.float16)
        # projections: fold gamma AND mean-centering (C = I - J/64) into
        # M/R so the PE consumes raw x; beta via M@beta / R@beta
        Cm = np.eye(D) - np.ones((D, D)) / D
        mgt = (Cm @ (Mf * gf[None, :]).T).astype(np.float16)  # (j, o)
        mbeta = (Mf @ bf).reshape(1, D).astype(np.float32)
        rgt = (Cm @ (Rf[i_set] * gf[None, :]).T).astype(np.float16)  # (j, 8)
        rbeta = (Rf[i_set] @ bf).reshape(1, 8).astype(np.float32)
        row32 = np.concatenate(
            [whatc.ravel(), vin.ravel(), mbeta.ravel(), rbeta.ravel(),
             np.array([math.pi / 2.0], dtype=np.float32)]
        ).astype(np.float32)
        bc32 = np.concatenate(
            [np.broadcast_to(row32, (128, row32.size)), kcols], axis=1
        )
        bc32 = np.ascontiguousarray(bc32)
        row16 = np.concatenate(
            [acol.ravel(), bcol.ravel(), p0.ravel()]
        ).astype(np.float16)
        bc16 = np.ascontiguousarray(np.broadcast_to(row16, (128, row16.size)))
        in_maps.append(
            {
                "x": x_sb,
                "u1h": u1h,
                "bc32": bc32,
                "bc16": bc16,
                "mgt": np.ascontiguousarray(mgt),
                "rgt": np.ascontiguousarray(rgt),
                "idm": idm,
            }
        )

    _CACHE["in_maps"] = in_maps
    res = run_bass_kernel_spmd(nc, in_maps, core_ids=list(range(N_CORES)))
    # y[p, (t i)] per core -> y_full[t, i_set]
    out = np.empty((NT, D), dtype=np.float32)
    for c in range(N_CORES):
        yc = res.results[c]["y"].reshape(128, NTG, 8)  # (p, tg, i_loc)
        i_set = np.arange(8) * 8 + c
        out[:, i_set] = yc.transpose(1, 0, 2).reshape(NT, 8)
    return out.reshape(B, S, D)

